# revision 3
# baseline (speedup 1.0000x reference)
"""Trainium2 Bass kernel for nn_CC_Decoder (hypernetwork-decoded per-pixel MLP).

Strategy (8 NeuronCores, data-parallel over batch: one sample per core):

Reference computation per sample:
  W_raw = conv1x1(x)                         # [1028, 256] channel matmul
  Wf    = W_raw @ wfine^T + wfine_b          # [1028, 256]
  layer j weights wj = Wf[257j : 257j+256], bias bj = Wf[257j+256]
  out = PE(coords)  -> 4 x (out @ wj + bj -> PReLU) -> last1 -> SiLU

Key algebraic optimization: the positional-encoding input x2 is an outer
sum over (y, x): x2[(y,x), :] = [u(y)(128) | v(x)(128)] with u = v = T
columns (T[f,t] = cos/sin(c_f * seq[t]) host table). Hence layer 0
  h0^T[c, (y,x)] = A^T[c, y] + B^T[c, x],
  A^T = w0_top^T @ T + b0,  B^T = w0_bot^T @ T   (two tiny 128x128x256 GEMMs)
so the 16384x256x256 layer-0 GEMM and the 16 MB x2 tensor are eliminated;
layer 0 becomes ACT-engine Prelu(B_chunk + A_col_as_bias).

Everything is kept feature-major: activations [256 feats -> 2x128
partitions, pixels free], so each layer is psum[c,px] += wj[k,c].T @
act[k,px], and per-layer PReLU+bias is a single ACT instruction per chunk
(bias rides the activation bias port; some chunks are offloaded to DVE via
prelu(h) = max(h, a*h) for engine balance). The last1 (256->3) matmuls
stack 4 pixel-tiles into one PSUM bank at 32-aligned partition offsets via
tile_position col-groups, amortizing SiLU to one instruction per 2048 px.

All matmul operands are bf16 (fp32 PSUM accumulation); measured end-to-end
relative error vs the fp32 reference is ~1e-3.
"""
import numpy as np
import ml_dtypes

bf16 = ml_dtypes.bfloat16

IMG = 128
NPX = IMG * IMG          # 16384 pixels
NF = 256                 # feature width
C1 = 1024                # conv in-channels
WD = 1028                # conv out-channels (= 4*257)
L = 4                    # generated layers
C2 = 3                   # output channels
TP = 512                 # pixel tile
NT = NPX // TP           # 32 tiles
M_ = 64
SIGMA = 10.0

_last_results = None     # stash for test.py introspection


def _host_tables():
    v0, v1 = -0.99999, 1.0
    r = (v1 - v0) / (2 * IMG)
    seq = v0 + r + 2 * r * np.arange(IMG, dtype=np.float64)
    j = np.arange(M_, dtype=np.float64)
    coeffs = 2.0 * np.pi * (SIGMA ** (j / M_))
    vp = coeffs[:, None] * seq[None, :]          # [64, 128]
    T = np.concatenate([np.cos(vp), np.sin(vp)], axis=0)  # [128, 128]
    return T.astype(np.float32)


def _build_program(alpha: float):
    import concourse.bass as bass
    import concourse.mybir as mybir
    import concourse.tile as tile
    import bir_patch_embedded  # installed below via sys.modules
    bir_patch_embedded.install()

    fp = mybir.dt.float32
    bf = mybir.dt.bfloat16
    PRELU = mybir.ActivationFunctionType.Prelu
    SILU = mybir.ActivationFunctionType.Silu
    ADD = mybir.AluOpType.add
    MULT = mybir.AluOpType.mult
    MAX = mybir.AluOpType.max

    # PReLU-on-DVE (max(h, a*h)) requires 0<=a<=1; otherwise keep all on ACT
    dve_ok = 0.0 <= alpha <= 1.0

    nc = bass.Bass()
    xb_d = nc.declare_dram_parameter("xb", [128, 8, NF], bf, isOutput=False)
    cwT_d = nc.declare_dram_parameter("cwT", [128, 8, WD], bf, isOutput=False)
    cb_d = nc.declare_dram_parameter("cb", [1, WD], bf, isOutput=False)
    wfT_d = nc.declare_dram_parameter("wfT", [128, 2, NF], bf, isOutput=False)
    wfb_d = nc.declare_dram_parameter("wfb", [1, NF], bf, isOutput=False)
    lwT_d = nc.declare_dram_parameter("lwT", [128, 2, C2], bf, isOutput=False)
    lbrep_d = nc.declare_dram_parameter("lbrep", [128, 1], fp, isOutput=False)
    Tt_d = nc.declare_dram_parameter("Tt", [128, 128], bf, isOutput=False)
    out_d = nc.declare_dram_parameter("out", [C2, NPX], fp, isOutput=True)
    out_r = out_d.rearrange("c (t x) -> c t x", x=TP)

    with tile.TileContext(nc) as tc:
        with (
            tc.tile_pool(name="wpool", bufs=1) as wp,
            tc.tile_pool(name="actp", bufs=3) as ap,
            tc.tile_pool(name="dvet", bufs=4) as dp,
            tc.tile_pool(name="outp", bufs=2) as op,
            tc.tile_pool(name="psmain", bufs=4, space="PSUM") as psm,
            tc.tile_pool(name="pslast", bufs=2, space="PSUM") as psl,
        ):
            # ---- persistent weights / tables ----
            xb = wp.tile([128, 8, NF], bf)
            cwT = wp.tile([128, 8, WD], bf)
            cb = wp.tile([1, WD], bf)
            wfT = wp.tile([128, 2, NF], bf)
            wfb = wp.tile([1, NF], bf)
            lwT = wp.tile([128, 2, C2], bf)
            lbrep = wp.tile([128, 1], fp)
            Tt = wp.tile([128, 128], bf)
            ones = wp.tile([1, 128], bf)
            Wt = wp.tile([128, 2, WD], bf)           # conv out, transposed (W^T)
            wj = [wp.tile([128, 2, NF], bf, tag=f"wj{j}", name=f"wj{j}") for j in range(L)]
            bjT = [wp.tile([128, 2], fp, tag=f"bj{j}", name=f"bj{j}") for j in range(L)]
            A_sb = wp.tile([128, 2, 128], fp)
            B_sb = wp.tile([128, 2, 128], bf)

            nc.sync.dma_start(xb[:], xb_d[:])
            for q in range(8):
                nc.sync.dma_start(cwT[:, q, :], cwT_d[:, q, :])
            nc.sync.dma_start(cb[:], cb_d[:])
            nc.sync.dma_start(wfT[:], wfT_d[:])
            nc.sync.dma_start(wfb[:], wfb_d[:])
            nc.sync.dma_start(lwT[:], lwT_d[:])
            nc.sync.dma_start(lbrep[:], lbrep_d[:])
            nc.sync.dma_start(Tt[:], Tt_d[:])
            nc.vector.memset(ones[:], 1.0)

            # ---- phase A: conv (1x1) -> W^T [hw=256 on 2 chunks, 1028 free] ----
            if True:
                psp = psm
                for m in range(2):
                    for off, sz in ((0, 512), (512, 512), (1024, 4)):
                        ps = psp.tile([128, 512], fp, tag="psmm", name="psA")
                        for q in range(8):
                            nc.tensor.matmul(
                                ps[:, :sz], xb[:, q, 128 * m:128 * (m + 1)],
                                cwT[:, q, off:off + sz],
                                start=(q == 0), stop=False)
                        nc.tensor.matmul(
                            ps[:, :sz], ones[:, 0:128], cb[:, off:off + sz],
                            start=False, stop=True)
                        nc.vector.tensor_copy(Wt[:, m, off:off + sz], ps[:, :sz])

                # ---- phase B: Wf rows -> per-layer weights + transposed biases ----
                for j in range(L):
                    r0 = 257 * j
                    for m in range(2):
                        ps = psp.tile([128, 512], fp, tag="psmm", name="psB")[:, :NF]
                        for k in range(2):
                            nc.tensor.matmul(
                                ps[:], Wt[:, k, r0 + 128 * m:r0 + 128 * (m + 1)],
                                wfT[:, k, :], start=(k == 0), stop=False)
                        nc.tensor.matmul(ps[:], ones[:, 0:128], wfb[:],
                                         start=False, stop=True)
                        nc.vector.tensor_copy(wj[j][:, m, :], ps[:])
                    for c in range(2):
                        psb = psp.tile([128, 512], fp, tag="psmm", name="psBb")[:, :1]
                        for k in range(2):
                            nc.tensor.matmul(
                                psb[:], wfT[:, k, 128 * c:128 * (c + 1)],
                                Wt[:, k, r0 + 256:r0 + 257],
                                start=(k == 0), stop=False)
                        nc.tensor.matmul(psb[:], wfb[:, 128 * c:128 * (c + 1)],
                                         ones[:, 0:1], start=False, stop=True)
                        nc.vector.tensor_copy(bjT[j][:, c:c + 1], psb[:])

                # ---- phase C: A = w0_top^T@T + b0 (f32), B = w0_bot^T@T (bf16) ----
                for c in range(2):
                    ps = psp.tile([128, 512], fp, tag="psmm", name="psC")[:, :128]
                    nc.tensor.matmul(ps[:], wj[0][:, 0, 128 * c:128 * (c + 1)],
                                     Tt[:], start=True, stop=True)
                    nc.vector.tensor_scalar(A_sb[:, c, :], ps[:],
                                            bjT[0][:, c:c + 1], None, ADD)
                    ps2 = psp.tile([128, 512], fp, tag="psmm", name="psC2")[:, :128]
                    nc.tensor.matmul(ps2[:], wj[0][:, 1, 128 * c:128 * (c + 1)],
                                     Tt[:], start=True, stop=True)
                    nc.vector.tensor_copy(B_sb[:, c, :], ps2[:])

            # prelu chunk engine schedule per (layer j in 1..3, chunk c):
            # True -> DVE 3-op path, False -> ACT single-op path
            on_dve = {(1, 1): dve_ok, (2, 1): dve_ok}

            # ---- main loop over pixel tiles ----
            accL = None
            for t in range(NT):
                g = t % 4
                act0 = ap.tile([128, 2, TP], bf, tag="act0")
                for c in range(2):
                    for gg in range(4):
                        y = 4 * t + gg
                        nc.scalar.activation(
                            act0[:, c, 128 * gg:128 * (gg + 1)], B_sb[:, c, :],
                            PRELU, bias=A_sb[:, c, y:y + 1], alpha=alpha)
                prev = act0
                for j in range(1, L):
                    actj = ap.tile([128, 2, TP], bf, tag=f"act{j}")
                    for c in range(2):
                        ps = psm.tile([128, TP], fp, tag="psmm")
                        for k in range(2):
                            nc.tensor.matmul(
                                ps[:], wj[j][:, k, 128 * c:128 * (c + 1)],
                                prev[:, k, :], start=(k == 0), stop=(k == 1))
                        if on_dve.get((j, c), False):
                            t1 = dp.tile([128, TP], bf, tag="dve1")
                            h1 = dp.tile([128, TP], bf, tag="dve2")
                            nc.vector.tensor_scalar(
                                t1[:], ps[:], bjT[j][:, c:c + 1], alpha, ADD, MULT)
                            nc.vector.tensor_scalar(
                                h1[:], ps[:], bjT[j][:, c:c + 1], None, ADD)
                            nc.vector.tensor_tensor(
                                actj[:, c, :], h1[:], t1[:], MAX)
                        else:
                            nc.scalar.activation(
                                actj[:, c, :], ps[:], PRELU,
                                bias=bjT[j][:, c:c + 1], alpha=alpha)
                    prev = actj
                # last1: stack 4 tiles in one PSUM bank at col-group offsets
                if g == 0:
                    accL = psl.tile([128, TP], fp, tag="pslastb")
                for k in range(2):
                    nc.tensor.matmul(
                        accL[32 * g:32 * g + C2, :], lwT[:, k, :], prev[:, k, :],
                        start=(k == 0), stop=(k == 1), tile_position=(0, 32 * g))
                if g == 3:
                    tb = t - 3
                    souf = op.tile([128, TP], fp, tag="souf")
                    nc.scalar.activation(souf[0:99, :], accL[0:99, :], SILU,
                                         bias=lbrep[0:99, 0:1])
                    for c in range(C2):
                        nc.sync.dma_start(out_r[c, tb:tb + 4, :],
                                          souf[c:c + 97:32, :])
    return nc


def kernel(x, conv_w, conv_b, wfine_w, wfine_b, last1_w, last1_b, prelu_a,
           **_ignored):
    global _last_results
    from concourse.bass_utils import run_bass_kernel_spmd

    x = np.asarray(x)
    B = x.shape[0]
    assert x.shape == (B, C1, 16, 16) and B == 8, x.shape

    conv_w = np.asarray(conv_w, np.float32)      # [1028, 1024]
    conv_b = np.asarray(conv_b, np.float32)      # [1028]
    wfine_w = np.asarray(wfine_w, np.float32)    # [256, 256]
    wfine_b = np.asarray(wfine_b, np.float32)    # [256]
    last1_w = np.asarray(last1_w, np.float32)    # [3, 256]
    last1_b = np.asarray(last1_b, np.float32)    # [3]
    alpha = float(np.asarray(prelu_a).reshape(-1)[0])

    # host-side shared operands (bf16)
    cwT = np.ascontiguousarray(
        conv_w.T.reshape(8, 128, WD).transpose(1, 0, 2)).astype(bf16)
    cb = conv_b.reshape(1, WD).astype(bf16)
    wfT = np.ascontiguousarray(
        wfine_w.T.reshape(2, 128, NF).transpose(1, 0, 2)).astype(bf16)
    wfb = wfine_b.reshape(1, NF).astype(bf16)
    lwT = np.ascontiguousarray(
        last1_w.T.reshape(2, 128, C2).transpose(1, 0, 2)).astype(bf16)
    lbrep = np.zeros((128, 1), np.float32)
    for g in range(4):
        lbrep[32 * g:32 * g + C2, 0] = last1_b
    Tt = _host_tables().astype(bf16)

    nc = _build_program(alpha)

    in_maps = []
    for b in range(B):
        xb = np.ascontiguousarray(
            x[b].reshape(8, 128, NF).transpose(1, 0, 2)).astype(bf16)
        in_maps.append({"xb": xb, "cwT": cwT, "cb": cb, "wfT": wfT,
                        "wfb": wfb, "lwT": lwT, "lbrep": lbrep, "Tt": Tt})

    res = run_bass_kernel_spmd(nc, in_maps, list(range(8)))
    _last_results = res
    out = np.stack([res.results[b]["out"].reshape(C2, IMG, IMG)
                    for b in range(B)])
    return out.astype(np.float32)


# ---------------------------------------------------------------------------
# Embedded walrus workaround (kernel.py must be self-contained): this walrus
# build accepts at most ONE sync wait per instruction; Tile attaches several.
# Split them into preceding single-wait NoOps at the BIR-JSON level, and make
# the TileContext tail drain emit one single-wait drain per logical proc.
# ---------------------------------------------------------------------------
import sys as _sys
import types as _types

_patch_mod = _types.ModuleType("bir_patch_embedded")
_patch_src = r'''
import json

def install():
    import concourse.bass_utils as _bu
    import concourse.bass2jax as _b2j
    import concourse.tile as _tile
    from concourse.vector_clock import ScopedClock, VectorClock

    if getattr(_bu, "_wait_legalizer_installed", False):
        return
    _bu._wait_legalizer_installed = True
    _orig_compile = _bu.compile_bir_kernel

    def _legalize_waits(bir_json):
        m = json.loads(bir_json)
        cnt = 0
        changed = False
        for fn in m.get("functions", []):
            for bb in fn.get("blocks", []):
                new_instrs = []
                for ins in bb.get("instructions", []):
                    si = ins.get("sync_info")
                    ow = (si or {}).get("on_wait") or []
                    if len(ow) > 1:
                        changed = True
                        for w in ow[:-1]:
                            cnt += 1
                            new_instrs.append({
                                "engine": ins["engine"],
                                "ins": [], "outs": [],
                                "name": "WSPLIT-%d" % cnt,
                                "opcode": "NoOp",
                                "sync_info": {"on_update": [], "on_wait": [w]},
                                "debug": ins.get("debug", 0),
                            })
                        si["on_wait"] = [ow[-1]]
                    new_instrs.append(ins)
                bb["instructions"] = new_instrs
        if not changed:
            return bir_json
        return json.dumps(m).encode()

    def _compile_legalized(bir_json, tmpdir, neff_name="file.neff"):
        return _orig_compile(_legalize_waits(bir_json), tmpdir, neff_name)

    _bu.compile_bir_kernel = _compile_legalized
    _b2j.compile_bir_kernel = _compile_legalized

    def _drain_and_barrier_split(self, tick_clock, wait_clock):
        nc = self.nc
        vclock = tick_clock.global_clock
        n = len(vclock)
        for p in range(n):
            t = vclock[p]
            if t <= 0:
                continue
            v = VectorClock([0] * n)
            v.require_at_least(p, t)
            d = nc.sync.drain()
            wait_clock.add_sem_waits(d.ins, ScopedClock({None: v}))
        nc.all_engine_barrier()
        popped = nc._tile_sem_poison_stack.pop()
        assert popped is self._sem_poison
        nc.clear_and_free_semaphores(list(self.sems.allocated().values()))
        nc.all_engine_barrier()

    _tile.TileContext._drain_and_barrier = _drain_and_barrier_split
'''
exec(_patch_src, _patch_mod.__dict__)
_sys.modules["bir_patch_embedded"] = _patch_mod
